# revision 2
# baseline (speedup 1.0000x reference)
"""CrossAttention kernel for Trainium2, 8-core data parallel.

ref: q = x@Wq; k,v = split(y@Wkv); dots[b,h] = (q_bh . k_bh)/64;
     attn = softmax_h(dots); out = attn[...,None]*v; res = out@Wproj + b

Per core (8192 rows): batch-major layout. Per 128-row tile:
  PE-transpose x,y -> xT,yT (stationary operands), fp32r matmuls for
  Q/K/V (N=512 moving weight slices), DVE dots + ACT exp(softmax,
  unnormalized) + DVE broadcast mul, PE-transpose OUT, proj matmul,
  fused (psum*recip)+bias eviction. Two-stage software pipeline keeps
  PE busy across the DVE/ACT softmax chain.
"""
import sys
sys.path.insert(0, "/opt/trn_rl_repo")
import numpy as np

import concourse.bass as bass
import concourse.mybir as mybir
import concourse.tile as tile
from concourse import bacc
from concourse.bass_utils import run_bass_kernel_spmd
from concourse.masks import make_identity

P = 128
B = 65536
DIM = 1024
NCORES = 8
BL = B // NCORES           # 8192 rows per core
NBT = BL // P              # 64 batch tiles
ND = DIM // P              # 8 contraction tiles
H, HD = 16, 64

f32 = mybir.dt.float32
f32r = mybir.dt.float32r
ExpF = mybir.ActivationFunctionType.Exp
MUL = mybir.AluOpType.mult
ADD = mybir.AluOpType.add

_NC = None


def _build():
    nc = bacc.Bacc(None, target_bir_lowering=False, debug=False)
    x_d = nc.dram_tensor("x", [BL, DIM], f32, kind="ExternalInput")
    y_d = nc.dram_tensor("y", [BL, DIM], f32, kind="ExternalInput")
    wq_d = nc.dram_tensor("wq", [P, ND, DIM], f32, kind="ExternalInput")
    wk_d = nc.dram_tensor("wk", [P, ND, DIM], f32, kind="ExternalInput")
    wv_d = nc.dram_tensor("wv", [P, ND, DIM], f32, kind="ExternalInput")
    wp_d = nc.dram_tensor("wp", [P, ND, DIM], f32, kind="ExternalInput")
    bias_d = nc.dram_tensor("bias", [P, DIM], f32, kind="ExternalInput")
    out_d = nc.dram_tensor("out", [BL, DIM], f32, kind="ExternalOutput")

    with tile.TileContext(nc) as tc:
        with (
            tc.tile_pool(name="const", bufs=1) as const,
            tc.tile_pool(name="wpool", bufs=1) as wpool,
            tc.tile_pool(name="xy", bufs=2) as xy,
            tc.tile_pool(name="tp", bufs=2) as tp,
            tc.tile_pool(name="mid", bufs=2) as mid,
            tc.tile_pool(name="sm", bufs=2) as sm,
            tc.tile_pool(name="qkp", bufs=1) as qkp,
            tc.tile_pool(name="pmm", bufs=6, space="PSUM") as pmm,
            tc.tile_pool(name="pst", bufs=2, space="PSUM") as pst,
        ):
            ident = const.tile([P, P], f32)
            make_identity(nc, ident)
            bias = const.tile([P, DIM], f32)
            nc.sync.dma_start(bias[:], bias_d[:])
            ws = {}
            for nm, dd in (("wq", wq_d), ("wk", wk_d), ("wv", wv_d),
                           ("wp", wp_d)):
                w = wpool.tile([P, ND, DIM], f32, tag=nm)
                nc.sync.dma_start(w[:].bitcast(f32r), dd[:].bitcast(f32r))
                ws[nm] = w

            def transpose_in(dst, src):
                # src [128, 1024] batch-major -> dst [128, 8, 128] f32r bytes
                for g in range(2):
                    pt = pst.tile([P, 4 * P], f32, tag="pt")
                    for i in range(4):
                        d = g * 4 + i
                        nc.tensor.transpose(
                            pt[:, i * P:(i + 1) * P],
                            src[:, d * P:(d + 1) * P], ident[:])
                    nc.scalar.copy(
                        dst[:, g * 4:(g + 1) * 4, :].bitcast(f32r), pt[:])

            def stage1(bt):
                xraw = xy.tile([P, DIM], f32, tag="x")
                nc.sync.dma_start(xraw[:], x_d[bt * P:(bt + 1) * P, :])
                yraw = xy.tile([P, DIM], f32, tag="y")
                nc.sync.dma_start(yraw[:], y_d[bt * P:(bt + 1) * P, :])
                xT = tp.tile([P, ND, P], f32, tag="xT")
                transpose_in(xT, xraw)
                yT = tp.tile([P, ND, P], f32, tag="yT")
                transpose_in(yT, yraw)

                psq = [pmm.tile([P, 512], f32, tag="mm", name=f"psq{i}")
                       for i in range(2)]
                psk = [pmm.tile([P, 512], f32, tag="mm", name=f"psk{i}")
                       for i in range(2)]
                psv = [pmm.tile([P, 512], f32, tag="mm", name=f"psv{i}")
                       for i in range(2)]
                for ps_list, wname, src in ((psq, "wq", xT), (psk, "wk", yT),
                                            (psv, "wv", yT)):
                    w = ws[wname]
                    for jh in range(2):
                        for d in range(ND):
                            nc.tensor.matmul(
                                ps_list[jh][:],
                                src[:, d, :].bitcast(f32r),
                                w[:, d, jh * 512:(jh + 1) * 512].bitcast(f32r),
                                start=(d == 0), stop=(d == ND - 1))
                ksb = mid.tile([P, DIM], f32, tag="k")
                for jh in range(2):
                    nc.scalar.copy(ksb[:, jh * 512:(jh + 1) * 512], psk[jh][:])
                qk = qkp.tile([P, DIM], f32, tag="qk")
                for jh in range(2):
                    nc.vector.tensor_tensor(
                        out=qk[:, jh * 512:(jh + 1) * 512], in0=psq[jh][:],
                        in1=ksb[:, jh * 512:(jh + 1) * 512], op=MUL)
                dots = sm.tile([P, H], f32, tag="dots")
                nc.vector.tensor_reduce(
                    out=dots[:], in_=qk[:].rearrange("p (h d) -> p h d", d=HD),
                    axis=mybir.AxisListType.X, op=ADD)
                edots = sm.tile([P, H], f32, tag="edots")
                esum = sm.tile([P, 1], f32, tag="esum")
                nc.scalar.activation(edots[:], dots[:], ExpF, scale=1.0 / 64.0,
                                     accum_out=esum[:])
                rec = sm.tile([P, 1], f32, tag="rec")
                nc.vector.reciprocal(rec[:], esum[:])
                outm = mid.tile([P, DIM], f32, tag="outm")
                for jh in range(2):
                    nc.vector.tensor_tensor(
                        out=outm[:, jh * 512:(jh + 1) * 512].rearrange(
                            "p (h d) -> p h d", d=HD),
                        in0=psv[jh][:].rearrange("p (h d) -> p h d", d=HD),
                        in1=edots[:, jh * 8:(jh + 1) * 8].unsqueeze(2)
                            .broadcast_to([P, 8, HD]),
                        op=MUL)
                return outm, rec

            def stage2(bt, outm, rec):
                outT = tp.tile([P, ND, P], f32, tag="outT")
                transpose_in(outT, outm)
                res = mid.tile([P, DIM], f32, tag="res")
                for nh in range(2):
                    pr = pmm.tile([P, 512], f32, tag="mm")
                    for j in range(ND):
                        nc.tensor.matmul(
                            pr[:], outT[:, j, :].bitcast(f32r),
                            ws["wp"][:, j, nh * 512:(nh + 1) * 512].bitcast(f32r),
                            start=(j == 0), stop=(j == ND - 1))
                    nc.vector.scalar_tensor_tensor(
                        out=res[:, nh * 512:(nh + 1) * 512], in0=pr[:],
                        scalar=rec[:], in1=bias[:, nh * 512:(nh + 1) * 512],
                        op0=MUL, op1=ADD)
                nc.sync.dma_start(out_d[bt * P:(bt + 1) * P, :], res[:])

            prev = None
            for bt in range(NBT):
                cur = stage1(bt)
                if prev is not None:
                    stage2(bt - 1, *prev)
                prev = cur
            stage2(NBT - 1, *prev)
    nc.compile()
    return nc


def _tile_w(W):
    return np.ascontiguousarray(
        W.astype(np.float32).reshape(ND, P, W.shape[1]).transpose(1, 0, 2))


def kernel(**inputs):
    global _NC
    x = np.ascontiguousarray(np.asarray(inputs["x"], np.float32))
    y = np.ascontiguousarray(np.asarray(inputs["y"], np.float32))
    Wq = np.asarray(inputs["Wq"], np.float32)
    Wkv = np.asarray(inputs["Wkv"], np.float32)
    Wp = np.asarray(inputs["Wproj"], np.float32)
    bp = np.asarray(inputs["bproj"], np.float32)
    wq, wk, wv, wp = (_tile_w(Wq), _tile_w(Wkv[:, :DIM]),
                      _tile_w(Wkv[:, DIM:]), _tile_w(Wp))
    biasf = np.ascontiguousarray(np.broadcast_to(bp, (P, DIM))).astype(np.float32)
    if _NC is None:
        _NC = _build()
    in_maps = [
        {"x": x[i * BL:(i + 1) * BL], "y": y[i * BL:(i + 1) * BL],
         "wq": wq, "wk": wk, "wv": wv, "wp": wp, "bias": biasf}
        for i in range(NCORES)
    ]
    res = run_bass_kernel_spmd(_NC, in_maps, list(range(NCORES)))
    return np.concatenate(
        [np.asarray(res.results[i]["out"], np.float32) for i in range(NCORES)],
        axis=0)
